# revision 38
# baseline (speedup 1.0000x reference)
"""Trainium2 Bass kernel for the AG_RoPE-style linear-attention encoder layer.

Distribution (8 NeuronCores): core c -> (batch = c//2, H-half = c%2).
Each core handles one [C, 64, 128] slab of x/out for its batch; the only
cross-core dependency is the KV/Ksum reduction over the batch's source
tokens -> [256, 129] f32 AllReduce over core pairs.

v3 design (v2 + engine rebalance, fewer/bigger PE instructions, overlap):
  - bf16 data path (PSUM f32), x resident in SBUF, as in v2.
  - maxpool = H-max tensor_tensor tree (DVE 2x bf16 / Pool split) + small
    W tensor_reduce, replacing the 5-D tensor_reduce (2194 -> ~1500 ns/slab).
  - single activation table (natural_log_exp_and_others): rstd = exp(-.5*
    ln(var+eps)); no LoadActFuncSet switches, no DVE reciprocal in chunks.
  - W-upsample written in natural token order -> MLP1 msg contribution is
    2 matmuls of N=512 per m-tile instead of 8 of N=128.
  - KV computed as full [128,128] token-contraction matmuls (diag head
    blocks extracted into a zeroed AllReduce payload) -> 8 matmuls, and
    zbmsg uses one block-diagonal [128,128] lhsT matmul per (ct, half).
  - epilogue fused: ocx = (tt + norm2_b) + x via Pool scalar_tensor_tensor
    (kills the x+b2 staging pass); LN2 row-scale tt reads rstd broadcast
    directly from PSUM.
  - emission order: source chain + collective send first; x pooling, LN1,
    Q (both halves) overlap the collective; half-1 z/merge tail and its
    chunks interleave so chunks 0..7 start as soon as half 0 is merged.
"""

import sys

for _p in ("/opt/trn_rl_repo",):
    if _p not in sys.path:
        sys.path.insert(0, _p)

import ml_dtypes
import numpy as np

import concourse.bass as bass
import concourse.mybir as mybir
import concourse.tile as tile
from concourse import bacc
from concourse.bass_utils import run_bass_kernel_spmd

F32 = mybir.dt.float32
BF16 = mybir.dt.bfloat16
ALU = mybir.AluOpType
ACTF = mybir.ActivationFunctionType
AX = mybir.AxisListType

C = 256
NH = 8
D = 32
EPS_LN = 1e-5
HL, WF = 64, 128          # rows per core, full width
PH, PW = 18, 34           # padded pooled grid (16+2 x 32+2)
NQ = PH * PW              # 612 q tokens
NS = (HL // 4) * (WF // 4)  # 512 source tokens
NTOK = HL * WF            # 8192 out tokens/core
CHUNK = 512
NCH = NTOK // CHUNK       # 16 chunks (4 rows each)
# bilinear x4 (half-pixel): out[4k+r] = pp[k+s] + w*(pp[k+s+1]-pp[k+s])
UPW = {0: (0, 0.625), 1: (0, 0.875), 2: (1, 0.125), 3: (1, 0.375)}
# x-side halves: (pool-row start, pool rows, token offset, tokens, hu k0)
HALVES = ((0, 10, 0, 340, 0), (10, 8, 340, 272, 8))


def _build_body(nc, tc, io, ctx, use_collective=True):
    xs, xh, src, out = io["xs"], io["xh"], io["src"], io["out"]
    v = nc.vector
    s = nc.scalar
    g = nc.gpsimd
    t = nc.tensor
    sy = nc.sync

    fixed = ctx.enter_context(tc.tile_pool(name="fixed", bufs=1))

    # ---- weights to SBUF: host pre-tiled, one DMA each, on the ACT queue
    w1s = fixed.tile([128, 2048], BF16)  # cols = 512*ict + oc
    w2s = fixed.tile([128, 1024], BF16)  # cols = 256*ict + oc
    wqs = fixed.tile([128, 512], BF16)   # cols = 256*ict + oc
    wks = fixed.tile([128, 512], BF16)
    wvs = fixed.tile([128, 512], BF16)
    wms = fixed.tile([128, 512], BF16)
    for ws, nm in ((w1s, "w1"), (w2s, "w2"), (wqs, "wq"), (wks, "wk"),
                   (wvs, "wv"), (wms, "wm")):
        s.dma_start(out=ws, in_=io[nm].ap())
    # per-channel vectors: cols = b2, ig2, b1, ig2^2  ([128, 4] per ctile)
    vec = fixed.tile([128, 8], F32)
    for ct in range(2):
        s.dma_start(out=vec[:, 4 * ct:4 * (ct + 1)],
                    in_=io["vecs"].ap()[128 * ct:128 * (ct + 1), :])
    b2 = [vec[:, 4 * ct + 0:4 * ct + 1] for ct in range(2)]
    ig2 = [vec[:, 4 * ct + 1:4 * ct + 2] for ct in range(2)]
    b1c = [vec[:, 4 * ct + 2:4 * ct + 3] for ct in range(2)]
    gat = fixed.tile([1, 256], BF16)   # row: g1 (free dim = channel)
    s.dma_start(out=gat, in_=io["gat"].ap())
    hb = fixed.tile([8, 256], BF16)    # per-head row->32-partition blocks
    s.dma_start(out=hb, in_=io["hbm"].ap())

    epsc = fixed.tile([1, 1], F32)
    v.memset(epsc, EPS_LN)
    ones_colF = fixed.tile([128, 1], F32)
    v.memset(ones_colF, 1.0)
    ones_col = fixed.tile([128, 1], BF16)
    s.copy(ones_col, ones_colF)
    ones_rowF = fixed.tile([1, 128], F32)
    v.memset(ones_rowF, 1.0)
    ones_row = fixed.tile([1, 128], BF16)
    s.copy(ones_row, ones_rowF)
    ksb = fixed.tile([128, 16], BF16)  # block-diag Ksum cols: 8*ct + head
    zf16 = fixed.tile([128, 16], F32)
    v.memset(zf16, 0.0)
    s.copy(ksb, zf16)

    # resident x: [128, 8192] bf16 per ctile (rows-major 64x128)
    xres = [fixed.tile([128, NTOK], BF16, name=f"xres{ct}")
            for ct in range(2)]

    dram = ctx.enter_context(tc.tile_pool(name="dram", bufs=1, space="DRAM"))
    cc_in = dram.tile([C, 129], F32)
    cc_out = dram.tile([C, 129], F32)

    bridge = ctx.enter_context(tc.tile_pool(name="bridge", bufs=1))
    hu = [bridge.tile([128, HL * PW], BF16, name=f"hu{ct}", tag=f"hu{ct}")
          for ct in range(2)]

    pa = ctx.enter_context(tc.tile_pool(name="pa", bufs=1))
    pb = ctx.enter_context(tc.tile_pool(name="pb", bufs=1))
    # one PSUM pool for everything: pp(5) + p2(3) = 8 banks
    pap = ctx.enter_context(tc.tile_pool(name="pap", bufs=1, space="PSUM"))

    def pp_tile(n=512):
        return pap.tile([128, 512], F32, name="ppt", tag="pp", bufs=5)[:, :n]

    def st_tile(p=1, n=512):
        return pap.tile([128, 512], F32, name="stt", tag="pp",
                        bufs=5)[:p, :n]

    # ---- maxpool helper: H-max tt tree (DVE 2x bf16) + W tensor_reduce.
    # Only DVE supports max / tensor_reduce, so pooling is DVE-only.
    # src16: [128, 16*WF] view of 16 input rows; dst: [p, 4, 32] view.
    def pool_slab(src16, dst, tg):
        s4 = src16.rearrange("p (x two w) -> p x two w", two=2, w=WF)
        h1t = pa.tile([128, 8 * WF], BF16, name="h1t", tag="h1p", bufs=2)
        h13 = h1t.rearrange("p (x w) -> p x w", w=WF)
        v.tensor_tensor(h13, s4[:, :, 0, :], s4[:, :, 1, :], ALU.max)
        h14 = h1t.rearrange("p (x two w) -> p x two w", two=2, w=WF)
        h2t = pa.tile([128, 4 * WF], BF16, name="h2t", tag="h2p", bufs=2)
        h23 = h2t.rearrange("p (x w) -> p x w", w=WF)
        v.tensor_tensor(h23, h14[:, :, 0, :], h14[:, :, 1, :], ALU.max)
        v.tensor_reduce(dst, h2t.rearrange("p (h pw dw) -> p h pw dw",
                                           h=4, dw=4), AX.X, ALU.max)

    # 4-row halo strip -> one pooled row ([p, 1, 32] dst view)
    def pool_halo(src4, dst, tg):
        s4 = src4.rearrange("p (x two w) -> p x two w", two=2, w=WF)
        h1t = pa.tile([128, 2 * WF], BF16, name="hh1", tag="hh1p", bufs=2)
        h13 = h1t.rearrange("p (x w) -> p x w", w=WF)
        v.tensor_tensor(h13, s4[:, :, 0, :], s4[:, :, 1, :], ALU.max)
        h14 = h1t.rearrange("p (x two w) -> p x two w", two=2, w=WF)
        h2t = pa.tile([128, WF], BF16, name="hh2", tag="hh2p", bufs=2)
        v.tensor_tensor(h2t, h14[:, 0, 0, :], h14[:, 0, 1, :], ALU.max)
        v.tensor_reduce(dst, h2t.rearrange("p (pw dw) -> p pw dw", dw=4),
                        AX.X, ALU.max)

    # ---------------- LN1 helper: out = t*A + B ---------------------------
    # A = g1 (x) rs, B = b1 (x) 1 + g1 (x) (-mu*rs); token stats via
    # ones-matmuls over the channel (partition) dim; rs = exp(-.5 ln(var+e)).
    def ln1(toks, o, n, outs):
        ps_sum = st_tile(1, n)
        ps_ssq = st_tile(1, n)
        for ct in range(2):
            tk = toks[ct][:, o:o + n]
            sq = pa.tile([128, 512], BF16, name="lsq", tag="lsq",
                         bufs=2)[:, :n]
            s.activation(sq, tk, ACTF.Square)
            t.matmul(ps_ssq, ones_col, sq, start=(ct == 0), stop=(ct == 1))
            t.matmul(ps_sum, ones_col, tk, start=(ct == 0), stop=(ct == 1))
        mu = pa.tile([1, 512], F32, name="mu", tag="mu", bufs=2)[:, :n]
        mu2 = pa.tile([1, 512], F32, name="mu2", tag="mu2", bufs=2)[:, :n]
        var = pa.tile([1, 512], F32, name="var", tag="var", bufs=2)[:, :n]
        std = pa.tile([1, 512], F32, name="std", tag="std", bufs=2)[:, :n]
        rs = pa.tile([1, 512], BF16, name="rs", tag="rs", bufs=2)[:, :n]
        nm = pa.tile([1, 512], BF16, name="nm", tag="nm", bufs=2)[:, :n]
        s.activation(mu, ps_sum, ACTF.Identity, scale=1.0 / C)
        s.activation(mu2, mu, ACTF.Square)
        v.scalar_tensor_tensor(var, ps_ssq, 1.0 / C, mu2, ALU.mult,
                               ALU.subtract)
        s.activation(std, var, ACTF.Sqrt, bias=epsc, scale=1.0)
        v.reciprocal(rs, std)
        v.scalar_tensor_tensor(nm, mu, -1.0, rs, ALU.mult, ALU.mult)
        for ct in range(2):
            a_b = pp_tile(n)
            b_b = pp_tile(n)
            t.matmul(a_b, gat[:, 128 * ct:128 * (ct + 1)], rs,
                     start=True, stop=True)
            t.matmul(b_b, gat[:, 128 * ct:128 * (ct + 1)], nm,
                     start=True, stop=True)
            tmp = pa.tile([128, 512], F32, name="lt1", tag="lt1",
                          bufs=2)[:, :n]
            v.tensor_tensor(tmp, toks[ct][:, o:o + n], a_b, ALU.mult)
            v.scalar_tensor_tensor(outs[ct][:, o:o + n], tmp, b1c[ct], b_b,
                                   ALU.add, ALU.add)

    # ---------------- phase S: source DMA + pooling -----------------------
    sp = [pa.tile([128, NS], BF16, name=f"sp{ct}", tag=f"sp{ct}")
          for ct in range(2)]
    for hc in range(4):  # 16 source rows -> 4 pooled rows per iteration
        for ct in range(2):
            sch = pa.tile([128, 16 * WF], BF16, name="sch", tag="sch", bufs=6)
            sy.dma_start(out=sch.rearrange("p (h w) -> p h w", h=16),
                         in_=src.ap()[128 * ct:128 * (ct + 1),
                                      16 * hc:16 * (hc + 1), :])
            dst = sp[ct][:, 128 * hc:128 * (hc + 1)].rearrange(
                "p (h w) -> p h w", h=4)
            pool_slab(sch, dst, f"s{ct}")

    s_ln = [pa.tile([128, NS], BF16, name=f"sln{ct}", tag=f"sln{ct}")
            for ct in range(2)]
    ln1(sp, 0, NS, s_ln)

    # ---------------- phase A2: K/V token-major projections + KV ----------
    kp_tm, v_tm = [], []
    for st_ in range(4):  # 128-token tiles of the 512 source tokens
        psk = pp_tile(256)
        psv = pp_tile(256)
        for kt in range(2):
            lt = s_ln[kt][:, 128 * st_:128 * (st_ + 1)]
            t.matmul(psk, lt, wks[:, 256 * kt:256 * (kt + 1)],
                     start=(kt == 0), stop=(kt == 1))
            t.matmul(psv, lt, wvs[:, 256 * kt:256 * (kt + 1)],
                     start=(kt == 0), stop=(kt == 1))
        kp = pa.tile([128, 256], BF16, name=f"kp{st_}", tag=f"kp{st_}")
        vv = pa.tile([128, 256], BF16, name=f"vv{st_}", tag=f"vv{st_}")
        mn = pa.tile([128, 512], F32, name="mn", tag="eluT", bufs=4)[:, :256]
        v.tensor_scalar(mn, psk, 0.0, None, ALU.min)  # min(x, 0) (DVE)
        ee = pa.tile([128, 512], BF16, name="ee", tag="eluE",
                     bufs=4)[:, :256]
        s.activation(ee, mn, ACTF.Exp)                # exp(min(x,0))
        rp = pa.tile([128, 512], BF16, name="rp", tag="eluR", bufs=4)[:, :256]
        s.activation(rp, psk, ACTF.Relu)              # max(x, 0)
        g.tensor_tensor(kp, rp, ee, ALU.add)          # elu(x)+1 (Pool)
        v.tensor_scalar(vv, psv, 0.0, None, ALU.add)
        kp_tm.append(kp)
        v_tm.append(vv)

    # full [128,128] KV (token contraction) + Ksum, one psum tile
    kvks = pap.tile([128, 512], F32, name="kvks", tag="pp", bufs=5)[:, :264]
    ps_kv = [kvks[:, 0:128], kvks[:, 132:260]]
    ps_ks = [kvks[:, 128:129], kvks[:, 260:261]]
    for ct in range(2):
        for st_ in range(4):
            t.matmul(ps_kv[ct], kp_tm[st_][:, 128 * ct:128 * (ct + 1)],
                     v_tm[st_][:, 128 * ct:128 * (ct + 1)],
                     start=(st_ == 0), stop=(st_ == 3),
                     skip_group_check=True)
            t.matmul(ps_ks[ct], kp_tm[st_][:, 128 * ct:128 * (ct + 1)],
                     ones_col, start=(st_ == 0), stop=(st_ == 3),
                     skip_group_check=True)
    # AllReduce payload: diag head blocks of KV + Ksum col, rest zero
    arb = [pa.tile([128, 129], F32, name=f"arb{ct}", tag=f"arb{ct}")
           for ct in range(2)]
    for ct in range(2):
        v.memset(arb[ct], 0.0)
        for hl in range(4):
            v.tensor_scalar(arb[ct][32 * hl:32 * (hl + 1),
                                    32 * hl:32 * (hl + 1)],
                            ps_kv[ct][32 * hl:32 * (hl + 1),
                                      32 * hl:32 * (hl + 1)],
                            0.0, None, ALU.add)
        v.tensor_scalar(arb[ct][:, 128:129], ps_ks[ct], 0.0, None, ALU.add)
        g.dma_start(out=cc_in[128 * ct:128 * (ct + 1), :], in_=arb[ct])

    if use_collective:
        g.collective_compute(
            "AllReduce", ALU.add,
            replica_groups=[[0, 1], [2, 3], [4, 5], [6, 7]],
            ins=[cc_in.opt()], outs=[cc_out.opt()])
    else:  # single-core cost-model build: plain copy stands in for AllReduce
        g.dma_start(out=cc_out.opt(), in_=cc_in.opt())

    # ---------------- phase A3: x pooling into padded grid ----------------
    # (overlaps the collective; PE-independent).  Top part (pool rows 0..12,
    # slabs 0..2 + top halo) feeds half 0; the bottom part is deferred into
    # the early chunk stream.
    xp = [pa.tile([128, NQ], BF16, name=f"xp{ct}", tag=f"xp{ct}")
          for ct in range(2)]
    v18 = [xp[ct].rearrange("p (h w) -> p h w", h=PH) for ct in range(2)]

    # x-side DMAs were deferred so the source stream owns early DMA bandwidth
    shh = []
    for ct in range(2):
        sh = pa.tile([128, 8 * WF], BF16, name=f"shh{ct}", tag=f"shh{ct}")
        sy.dma_start(out=sh.rearrange("p (h w) -> p h w", h=8),
                     in_=xh.ap()[128 * ct:128 * (ct + 1), :, :])
        shh.append(sh)
    for xi in range(4):
        for ct in range(2):
            sy.dma_start(
                out=xres[ct][:, 2048 * xi:2048 * (xi + 1)].rearrange(
                    "p (h w) -> p h w", h=16),
                in_=xs.ap()[128 * ct:128 * (ct + 1),
                            16 * xi:16 * (xi + 1), :])

    def xpool_top():
        for ct in range(2):
            pool_halo(shh[ct][:, 0:512], v18[ct][:, 0:1, 1:33], f"t{ct}")
            for xi in range(3):
                pool_slab(xres[ct][:, 2048 * xi:2048 * (xi + 1)],
                          v18[ct][:, 1 + 4 * xi:5 + 4 * xi, 1:33], f"x{ct}")
            # pooled col halos clamp-duplicate the adjacent pooled col
            g.tensor_scalar(v18[ct][:, 0:13, 0:1], v18[ct][:, 0:13, 1:2],
                            0.0, None, ALU.add)
            g.tensor_scalar(v18[ct][:, 0:13, 33:34], v18[ct][:, 0:13, 32:33],
                            0.0, None, ALU.add)

    def xpool_bottom():
        for ct in range(2):
            pool_slab(xres[ct][:, 2048 * 3:2048 * 4],
                      v18[ct][:, 13:17, 1:33], f"x{ct}")
            pool_halo(shh[ct][:, 512:1024], v18[ct][:, 17:18, 1:33],
                      f"b{ct}")
            g.tensor_scalar(v18[ct][:, 13:18, 0:1], v18[ct][:, 13:18, 1:2],
                            0.0, None, ALU.add)
            g.tensor_scalar(v18[ct][:, 13:18, 33:34],
                            v18[ct][:, 13:18, 32:33], 0.0, None, ALU.add)

    # ------- phase A4 state (x side, two token-halves) --------------------
    p_ln = [pa.tile([128, NQ], BF16, name=f"pln{ct}", tag=f"pln{ct}")
            for ct in range(2)]
    qp = [pa.tile([128, NQ], BF16, name=f"qp{ct}", tag=f"qp{ct}")
          for ct in range(2)]
    zz = pa.tile([8, NQ], BF16, name="zz", tag="zz")
    mz = [pa.tile([128, NQ], BF16, name=f"mz{ct}", tag=f"mz{ct}")
          for ct in range(2)]
    mp = [pa.tile([128, NQ], BF16, name=f"mp{ct}", tag=f"mp{ct}")
          for ct in range(2)]
    dh = [pa.tile([128, (PH - 1) * PW], BF16, name=f"dh{ct}", tag=f"dh{ct}")
          for ct in range(2)]

    def a_ln1(hs):
        pr0, pn, o, n, k0 = HALVES[hs]
        ln1(xp, o, n, p_ln)

    def a_q(hs):
        pr0, pn, o, n, k0 = HALVES[hs]
        for ct in range(2):
            psq = pp_tile(n)
            for kt in range(2):
                t.matmul(psq,
                         wqs[:, 256 * kt + 128 * ct:
                             256 * kt + 128 * ct + 128],
                         p_ln[kt][:, o:o + n],
                         start=(kt == 0), stop=(kt == 1))
            mn = pa.tile([128, 512], F32, name="qmn", tag="eluT",
                         bufs=4)[:, :n]
            v.tensor_scalar(mn, psq, 0.0, None, ALU.min)
            ee = pa.tile([128, 512], BF16, name="qee", tag="eluE",
                         bufs=4)[:, :n]
            s.activation(ee, mn, ACTF.Exp)
            rp = pa.tile([128, 512], BF16, name="qrp", tag="eluR",
                         bufs=4)[:, :n]
            s.activation(rp, psq, ACTF.Relu)
            g.tensor_tensor(qp[ct][:, o:o + n], rp, ee, ALU.add)

    def a_z(hs):
        pr0, pn, o, n, k0 = HALVES[hs]
        ps_z = st_tile(8, n)
        for ct in range(2):
            t.matmul(ps_z, ksb[:, 8 * ct:8 * (ct + 1)],
                     qp[ct][:, o:o + n], start=(ct == 0), stop=(ct == 1))
        v.reciprocal(zz[:, o:o + n], ps_z)

    def a_zbmsg(hs):
        pr0, pn, o, n, k0 = HALVES[hs]
        for ct in range(2):
            ps_zb = pp_tile(n)
            t.matmul(ps_zb, hb[:, 128 * ct:128 * (ct + 1)],
                     zz[:, o:o + n], start=True, stop=True)
            zbs = pa.tile([128, 512], BF16, name="zbs", tag="zbs",
                          bufs=2)[:, :n]
            s.copy(zbs, ps_zb)
            ps_m = pp_tile(n)
            t.matmul(ps_m, arbq[ct], qp[ct][:, o:o + n],
                     start=True, stop=True)
            v.tensor_tensor(mz[ct][:, o:o + n], ps_m, zbs, ALU.mult)

    def a_merge_hu(hs):
        pr0, pn, o, n, k0 = HALVES[hs]
        d0, dn = (0, 9) if hs == 0 else (9, 8)
        for ct in range(2):
            ps_g = pp_tile(n)
            for kt in range(2):
                t.matmul(ps_g,
                         wms[:, 256 * kt + 128 * ct:
                             256 * kt + 128 * ct + 128],
                         mz[kt][:, o:o + n],
                         start=(kt == 0), stop=(kt == 1))
            s.copy(mp[ct][:, o:o + n], ps_g)
            m3 = mp[ct].rearrange("p (h w) -> p h w", h=PH)
            d3 = dh[ct].rearrange("p (h w) -> p h w", h=PH - 1)
            v.tensor_tensor(d3[:, d0:d0 + dn, :],
                            m3[:, d0 + 1:d0 + dn + 1, :],
                            m3[:, d0:d0 + dn, :], ALU.subtract)
            hu4 = hu[ct].rearrange("p (k r w) -> p k r w", r=4, w=PW)
            for r in range(4):
                sh, w = UPW[r]
                v.scalar_tensor_tensor(hu4[:, k0:k0 + 8, r, :],
                                       d3[:, k0 + sh:k0 + sh + 8, :], w,
                                       m3[:, k0 + sh:k0 + sh + 8, :],
                                       ALU.mult, ALU.add)

    # ------- phase B chunk (W-upsample + MLP + LN2 + residual) ------------
    xb2_box = [None, None]
    mf_box = [None, None]

    def emit_chunk(kc):
        xc = [xres[ct][:, CHUNK * kc:CHUNK * (kc + 1)] for ct in range(2)]
        if kc % 4 == 0:  # stage x + norm2_b for the next 4 chunks (Pool)
            xi = kc // 4
            for ct in range(2):
                xb2_box[ct] = pb.tile([128, 2048], BF16, name="xb2",
                                      tag=f"xb2_{ct}", bufs=2)
                g.tensor_scalar(xb2_box[ct],
                                xres[ct][:, 2048 * xi:2048 * (xi + 1)],
                                b2[ct], None, ALU.add)

        if kc % 2 == 0:  # natural-order W-upsample, pair-batched (8 rows)
            for ct in range(2):
                hu3 = hu[ct].rearrange("p (h w) -> p h w", h=HL)
                hus = hu3[:, 4 * kc:4 * kc + 8, :]
                dw = pb.tile([128, 8 * 33], BF16, name="dw", tag=f"dw{ct}",
                             bufs=2)
                dw3 = dw.rearrange("p (h w) -> p h w", h=8)
                v.tensor_tensor(dw3, hus[:, :, 1:34], hus[:, :, 0:33],
                                ALU.subtract)
                mt = pb.tile([128, 2 * CHUNK], BF16, name="mfp",
                             tag=f"mf{ct}", bufs=2)
                m4 = mt.rearrange("p (h k r) -> p h k r", h=8, k=32, r=4)
                for r in range(4):
                    sh, w = UPW[r]
                    v.scalar_tensor_tensor(m4[:, :, :, r],
                                           dw3[:, :, sh:sh + 32], w,
                                           hus[:, :, sh:sh + 32],
                                           ALU.mult, ALU.add)
                mf_box[ct] = mt
        mf = [mf_box[ct][:, CHUNK * (kc % 2):CHUNK * (kc % 2) + CHUNK]
              for ct in range(2)]

        # MLP layer 1 + relu (relu rotated ACT/DVE/Pool/ACT)
        h1 = pb.tile([128, 2048], BF16, name="h1", tag="h1", bufs=2)
        for m in range(4):
            ps1 = pp_tile()
            for kt in range(2):
                t.matmul(ps1,
                         w1s[:, 512 * kt + 128 * m:
                             512 * kt + 128 * m + 128],
                         xc[kt], start=(kt == 0), stop=False)
            for ct in range(2):
                t.matmul(ps1,
                         w1s[:, 512 * (2 + ct) + 128 * m:
                             512 * (2 + ct) + 128 * m + 128],
                         mf[ct], start=False, stop=(ct == 1))
            hslice = h1[:, 512 * m:512 * m + 512]
            if m % 2 == 1:
                v.tensor_scalar(hslice, ps1, 0.0, None, ALU.max)
            else:
                s.activation(hslice, ps1, ACTF.Relu)

        # MLP layer 2 (W2 pre-centered * g2), per-m psum tiles
        ps2 = []
        for m in range(2):
            p2 = pap.tile([128, CHUNK], F32, name="p2", tag="p2", bufs=3)
            for kt in range(4):
                t.matmul(p2,
                         w2s[:, 256 * kt + 128 * m:
                             256 * kt + 128 * m + 128],
                         h1[:, 512 * kt:512 * kt + 512],
                         start=(kt == 0), stop=(kt == 3))
            ps2.append(p2)

        # LN2: rstd = rsqrt(mean((ps2*ig2)^2) + eps); std broadcast via PE,
        # reciprocal applied on the broadcast (saves a separate copy)
        ps_ss = st_tile(1)
        for ct in range(2):
            sqc = pb.tile([128, CHUNK], BF16, name="sqc", tag=f"sq{ct}",
                          bufs=2)
            s.activation(sqc, ps2[ct], ACTF.Square, scale=ig2[ct])
            t.matmul(ps_ss, ones_col, sqc, start=(ct == 0), stop=(ct == 1))
        std2 = pb.tile([1, CHUNK], BF16, name="std2", tag="std2", bufs=2)
        s.activation(std2, ps_ss, ACTF.Sqrt, bias=epsc, scale=1.0 / C)
        ps_rb = pap.tile([128, CHUNK], F32, name="p2", tag="p2", bufs=3)
        t.matmul(ps_rb, ones_row, std2, start=True, stop=True)
        rsb = pb.tile([128, CHUNK], BF16, name="rsb", tag="rsb", bufs=2)
        v.reciprocal(rsb, ps_rb)

        for ct in range(2):
            tt = pb.tile([128, CHUNK], BF16, name="tt", tag="tt", bufs=2)
            v.tensor_tensor(tt, ps2[ct], rsb, ALU.mult)
            # ocx <- tt + (x + b2) on the Pool engine, then DMA out (bf16)
            ocx = pb.tile([128, CHUNK], BF16, name="ocx", tag=f"oc{ct}",
                          bufs=2)
            g.tensor_tensor(ocx, tt,
                            xb2_box[ct][:, CHUNK * (kc % 4):
                                        CHUNK * (kc % 4) + CHUNK], ALU.add)
            sy.dma_start(out=out.ap()[128 * ct:128 * (ct + 1),
                                      4 * kc:4 * kc + 4, :],
                         in_=ocx.rearrange("p (h w) -> p h w", h=4))

    # ---- head: top x pooling, half-0 LN1/Q (overlaps the collective) -----
    xpool_top()
    a_ln1(0)
    a_q(0)

    # ---- collective receive ----------------------------------------------
    arbr = [pa.tile([128, 129], F32, name=f"arbr{ct}", tag=f"arbr{ct}")
            for ct in range(2)]
    arbq = [pa.tile([128, 128], BF16, name=f"arbq{ct}", tag=f"arbq{ct}")
            for ct in range(2)]
    for ct in range(2):
        g.dma_start(out=arbr[ct], in_=cc_out[128 * ct:128 * (ct + 1), :])
        v.tensor_scalar(arbq[ct], arbr[ct][:, 0:128], 0.0, None, ALU.add)
        for hl in range(4):
            v.tensor_scalar(ksb[32 * hl:32 * (hl + 1),
                                12 * ct + hl:12 * ct + hl + 1],
                            arbr[ct][32 * hl:32 * (hl + 1), 128:129],
                            0.0, None, ALU.add)

    # ---- half-0 tail, then chunks; half-1 work spread under chunks 0..4 --
    a_z(0)
    a_zbmsg(0)
    a_merge_hu(0)
    emit_chunk(0)
    xpool_bottom()
    emit_chunk(1)
    a_ln1(1)
    emit_chunk(2)
    a_q(1)
    emit_chunk(3)
    a_z(1)
    a_zbmsg(1)
    emit_chunk(4)
    a_merge_hu(1)
    for kc in range(5, NCH):
        emit_chunk(kc)


def build(use_collective=True):
    nc = bacc.Bacc("TRN2", target_bir_lowering=False, debug=False,
                   num_devices=8 if use_collective else 1)
    io = {
        "xs": nc.dram_tensor("xs", [C, HL, WF], BF16, kind="ExternalInput"),
        "xh": nc.dram_tensor("xh", [C, 8, WF], BF16, kind="ExternalInput"),
        "src": nc.dram_tensor("src", [C, HL, WF], BF16,
                              kind="ExternalInput"),
        "wq": nc.dram_tensor("wq", [128, 512], BF16, kind="ExternalInput"),
        "wk": nc.dram_tensor("wk", [128, 512], BF16, kind="ExternalInput"),
        "wv": nc.dram_tensor("wv", [128, 512], BF16, kind="ExternalInput"),
        "wm": nc.dram_tensor("wm", [128, 512], BF16, kind="ExternalInput"),
        "w1": nc.dram_tensor("w1", [128, 2048], BF16, kind="ExternalInput"),
        "w2": nc.dram_tensor("w2", [128, 1024], BF16, kind="ExternalInput"),
        "vecs": nc.dram_tensor("vecs", [C, 4], F32, kind="ExternalInput"),
        "gat": nc.dram_tensor("gat", [1, 256], BF16, kind="ExternalInput"),
        "hbm": nc.dram_tensor("hbm", [8, C], BF16, kind="ExternalInput"),
        "out": nc.dram_tensor("out", [C, HL, WF], BF16,
                              kind="ExternalOutput"),
    }
    from contextlib import ExitStack
    with tile.TileContext(nc) as tc:
        with ExitStack() as ctx:
            ctx.enter_context(nc.allow_low_precision(
                reason="bf16 data path is intentional; psum stays f32"))
            _build_body(nc, tc, io, ctx, use_collective=use_collective)
    nc.compile()
    return nc


_NC = None


def _get_nc():
    global _NC
    if _NC is None:
        _NC = build()
    return _NC


def make_in_maps(inputs):
    bfd = ml_dtypes.bfloat16
    x = np.asarray(inputs["x"], np.float32).astype(bfd)
    src = np.asarray(inputs["source"], np.float32).astype(bfd)
    g1 = np.asarray(inputs["norm1_g"], np.float32)
    b1 = np.asarray(inputs["norm1_b"], np.float32)
    g2 = np.asarray(inputs["norm2_g"], np.float32)
    b2 = np.asarray(inputs["norm2_b"], np.float32)
    w2 = np.asarray(inputs["mlp_w2"], np.float32)
    w2pp = (w2 - w2.mean(1, keepdims=True)) * g2[None, :]
    hbm = np.zeros((8, C), np.float32)
    for ct in range(2):
        for hl in range(4):
            hbm[4 * ct + hl, 128 * ct + 32 * hl:128 * ct + 32 * hl + 32] = 1.0
    ig2 = np.where(np.abs(g2) < 1e-12, 1.0, 1.0 / np.where(g2 == 0, 1, g2)
                   ).astype(np.float32)
    vecs = np.ascontiguousarray(np.stack([b2, ig2, b1, ig2 * ig2], axis=1))
    gat = np.ascontiguousarray(g1[None, :]).astype(bfd)

    def wtile(w, nt):
        w = np.asarray(w, np.float32)
        return np.ascontiguousarray(
            w.reshape(nt, 128, w.shape[1]).transpose(1, 0, 2).reshape(
                128, -1)).astype(bfd)

    shared = {
        "wq": wtile(inputs["w_q"], 2),
        "wk": wtile(inputs["w_k"], 2),
        "wv": wtile(inputs["w_v"], 2),
        "wm": wtile(inputs["w_merge"], 2),
        "w1": wtile(inputs["mlp_w1"], 4),
        "w2": wtile(w2pp, 4),
        "vecs": vecs, "gat": gat, "hbm": hbm.astype(bfd),
    }
    in_maps = []
    for core in range(8):
        b, half = core // 2, core % 2
        r0 = 64 * half
        xb = x[b]
        t0, bt0 = (0, 64) if half == 0 else (60, 124)
        xhs = np.ascontiguousarray(
            np.concatenate([xb[:, t0:t0 + 4, :], xb[:, bt0:bt0 + 4, :]],
                           axis=1))
        in_maps.append({
            "xs": np.ascontiguousarray(xb[:, r0:r0 + 64]),
            "xh": xhs,
            "src": np.ascontiguousarray(src[b][:, r0:r0 + 64]),
            **shared,
        })
    return in_maps


def run(inputs, **kwargs):
    nc = _get_nc()
    res = run_bass_kernel_spmd(nc, make_in_maps(inputs),
                               core_ids=list(range(8)), **kwargs)
    out = np.empty((4, C, 128, 128), np.float32)
    for core in range(8):
        b, half = core // 2, core % 2
        out[b, :, 64 * half:64 * half + 64] = np.asarray(
            res.results[core]["out"], dtype=np.float32)
    return out, res


def kernel(**inputs):
    out, _ = run(inputs)
    return out
